# revision 24
# baseline (speedup 1.0000x reference)
"""Multi-head attention (B=2, S=2048, DIM=512, H=8) on 8 Trainium2 cores.

Sharding: data-parallel over batch x tensor-parallel over heads.
Core c handles batch b = c // 4 and heads {2g, 2g+1} where g = c % 4
(i.e. output feature columns [128g : 128g+128]).  All sharding /
gathering happens host-side; no on-device collectives.

Per-core kernel. All matmul inputs are fp16 (10-bit mantissa, same
multiplier precision as TF32 for this N(0,1)-scaled data, but runs on
the fast normal PE path with fp32 PSUM accumulation); everything else
(PSUM, softmax denominators, normalization, output) stays fp32.

  - inputs arrive host-pretransposed as X^T [512, 2048] fp16; input
    DMAs are chunked by 512 seq columns so compute starts early,
  - Q^T, K^T projections in [out_dim(128), seq] layout (head h at
    partitions 64h..64h+63) - attention-ready; V in natural
    [seq, out_dim] tiles with a ones column per head so the ctx
    matmul also accumulates the softmax denominator for free,
  - scores^T = K_h @ Q_h^T per 128-row key tile (K=64), exp on
    ScalarE with the 1/sqrt(512) scale fused ([128,1024] tiles,
    fp16 output),
  - ctx^T accumulated over key tiles (lhsT = V tile [128,65],
    rhs = exp-scores [128,512], fp32 PSUM),
  - PE transpose back to natural layout, reciprocal + scale, DMA out.
"""

import os

import ml_dtypes
import numpy as np

DIM = 512
NUM_HEADS = 8
D_HEAD = 64
B = 2
S = 2048
N_CORES = 8
P = 128  # partitions
NK = DIM // P  # 4 contraction tiles for projections
NT = S // P  # 16 key tiles
VSTRIDE = 132  # V tile stride: [h0(64) | ones | h1(64) | 3 pad]
SCALE = float(1.0 / np.sqrt(512.0))
CH = 512  # input DMA / projection chunk (columns of seq)

_CACHE = {}


def _build_program():
    import concourse.tile as tile
    from concourse import bacc, mybir

    f32 = mybir.dt.float32
    f16 = mybir.dt.float16
    nc = bacc.Bacc("TRN2", target_bir_lowering=False, debug=False)

    io = {}
    io["xqT"] = nc.dram_tensor("xqT", [DIM, S], f16, kind="ExternalInput").ap()
    io["xkT"] = nc.dram_tensor("xkT", [DIM, S], f16, kind="ExternalInput").ap()
    io["xvT"] = nc.dram_tensor("xvT", [DIM, S], f16, kind="ExternalInput").ap()
    io["wq"] = nc.dram_tensor("wq", [P, DIM], f16, kind="ExternalInput").ap()
    io["wk"] = nc.dram_tensor("wk", [P, DIM], f16, kind="ExternalInput").ap()
    io["wv"] = nc.dram_tensor("wv", [P, DIM], f16, kind="ExternalInput").ap()
    io["bq2"] = nc.dram_tensor("bq2", [P, 1], f32, kind="ExternalInput").ap()
    io["bk2"] = nc.dram_tensor("bk2", [P, 1], f32, kind="ExternalInput").ap()
    io["bvb"] = nc.dram_tensor("bvb", [P, P], f32, kind="ExternalInput").ap()
    io["ident"] = nc.dram_tensor("ident", [P, P], f32, kind="ExternalInput").ap()
    io["out"] = nc.dram_tensor("out", [S, P], f32, kind="ExternalOutput").ap()

    with tile.TileContext(nc) as tc:
        _emit(tc, mybir, io)
    nc.compile()
    return nc


def _emit(tc, mybir, io):
    from contextlib import ExitStack

    nc = tc.nc
    f32 = mybir.dt.float32
    f16 = mybir.dt.float16
    Exp = mybir.ActivationFunctionType.Exp

    mm = nc.tensor.matmul

    with ExitStack() as ctx:
        const = ctx.enter_context(tc.tile_pool(name="const", bufs=1))
        qk = ctx.enter_context(tc.tile_pool(name="qk", bufs=1))
        vpool = ctx.enter_context(tc.tile_pool(name="vpool", bufs=1))
        csbpool = ctx.enter_context(tc.tile_pool(name="csbp", bufs=2))
        osmall = ctx.enter_context(tc.tile_pool(name="osmall", bufs=4))

        # constants
        wq_sb = const.tile([P, DIM], f16, tag="wq")
        wk_sb = const.tile([P, DIM], f16, tag="wk")
        wv_sb = const.tile([P, DIM], f16, tag="wv")
        bq_sb = const.tile([P, 1], f32, tag="bq")
        bk_sb = const.tile([P, 1], f32, tag="bk")
        bvb_sb = const.tile([P, P], f32, tag="bvb")
        id_sb = const.tile([P, P], f32, tag="ident")
        nc.sync.dma_start(wq_sb[:], io["wq"][:])
        nc.sync.dma_start(wk_sb[:], io["wk"][:])
        nc.sync.dma_start(wv_sb[:], io["wv"][:])
        nc.sync.dma_start(bq_sb[:], io["bq2"][:])
        nc.sync.dma_start(bk_sb[:], io["bk2"][:])
        nc.sync.dma_start(bvb_sb[:], io["bvb"][:])
        nc.sync.dma_start(id_sb[:], io["ident"][:])

        # persistent projection outputs
        QT = qk.tile([P, S], f16, tag="QT")  # [out_dim, seq]
        KT = qk.tile([P, S], f16, tag="KT")
        V = vpool.tile([P, NT * VSTRIDE], f16, tag="V")  # 16 x [128, 132]

        # ---- interleaved projections + attention ----
        # Attention head-pair structure: the two K=64 score matmuls per
        # key tile target disjoint PE row groups (partitions 0-63 /
        # 64-127) so they run concurrently AND register as full-array
        # activity for the HAM clock governor (unpacked K=64 streams
        # never un-throttle the PE).  ctx matmuls are K=128/M=65 which
        # hold the warm clock.
        #
        # Emission order pipelines the projection chunks against the
        # first query-chunk's attention (PE executes in program order,
        # so attention t-block c follows projection chunk c).
        with (
            tc.tile_pool(name="xin", bufs=24) as xin,
            tc.tile_pool(name="psq", bufs=2, space="PSUM") as psq,
            tc.tile_pool(name="pss", bufs=2, space="PSUM") as pss,
            tc.tile_pool(name="psc", bufs=1, space="PSUM") as psc,
            tc.tile_pool(name="es", bufs=3) as espool,
        ):

            def proj_chunk(c):
                cs = slice(c * CH, (c + 1) * CH)
                xt = {}
                for name, key in (("q", "xqT"), ("k", "xkT"), ("v", "xvT")):
                    ts = []
                    for k in range(NK):
                        tl = xin.tile([P, CH], f16, tag="xt", name="xt")
                        nc.sync.dma_start(tl[:], io[key][k * P : (k + 1) * P, cs])
                        ts.append(tl)
                    xt[name] = ts
                for name, w_sb, b_sb, dst in (
                    ("q", wq_sb, bq_sb, QT),
                    ("k", wk_sb, bk_sb, KT),
                ):
                    ps = psq.tile([P, CH], f32, tag="psq", name="psq")
                    for k in range(NK):
                        mm(
                            ps[:],
                            w_sb[:, k * P : (k + 1) * P],
                            xt[name][k][:, :],
                            start=(k == 0),
                            stop=(k == NK - 1),
                        )
                    nc.vector.tensor_scalar_add(dst[:, cs], ps[:], b_sb[:, 0:1])
                # V natural-layout tiles for this chunk (+ones column)
                for tl_i in range(CH // P):
                    ti = c * (CH // P) + tl_i
                    lsl = slice(tl_i * P, (tl_i + 1) * P)
                    ps = psq.tile([P, P], f32, tag="psq", name="psv")
                    for k in range(NK):
                        mm(
                            ps[:],
                            xt["v"][k][:, lsl],
                            wv_sb[:, k * P : (k + 1) * P],
                            start=(k == 0),
                            stop=(k == NK - 1),
                        )
                    o = ti * VSTRIDE
                    nc.vector.memset(V[:, o + 64 : o + 65], 1.0)
                    nc.vector.tensor_add(
                        V[:, o : o + 64], ps[:, 0:64], bvb_sb[:, 0:64]
                    )
                    nc.vector.tensor_add(
                        V[:, o + 65 : o + 129], ps[:, 64:128], bvb_sb[:, 64:128]
                    )

            def attn_block(q, cps, t0, t1):
                qs = slice(q * 512, (q + 1) * 512)
                for t in range(t0, t1):
                    sps = pss.tile([P, 1024], f32, tag="sps", name="sps")
                    for h in range(2):
                        hp = 64 * h
                        mm(
                            sps[:, h * 512 : (h + 1) * 512],
                            KT[hp : hp + 64, t * P : (t + 1) * P],
                            QT[hp : hp + 64, qs],
                            start=True,
                            stop=True,
                        )
                    es = espool.tile([P, 1024], f16, tag="es", name="es")
                    nc.scalar.activation(es[:], sps[:], Exp, scale=SCALE)
                    for h in range(2):
                        vo = t * VSTRIDE + 64 * h
                        mm(
                            cps[h][:],
                            V[:, vo : vo + 65],
                            es[:, h * 512 : (h + 1) * 512],
                            start=(t == 0),
                            stop=(t == NT - 1),
                        )

            def attn_tail(q, cps):
                # evacuate ctx^T, transpose back, normalize, store
                for h in range(2):
                    hp = 64 * h
                    sumcol = 64 if h == 0 else 0
                    csb = csbpool.tile([65, 512], f32, tag="csb", name="csb")
                    nc.vector.tensor_copy(csb[:], cps[h][:])
                    for u in range(4):
                        # transposes borrow the scores-PSUM slots
                        tp = pss.tile([P, 65], f32, tag="sps", name="tp")
                        nc.tensor.transpose(
                            tp[:], csb[:, u * P : (u + 1) * P], id_sb[0:65, 0:65]
                        )
                        r = osmall.tile([P, 1], f32, tag="recip", name="r")
                        nc.vector.reciprocal(r[:], tp[:, sumcol : sumcol + 1])
                        o = osmall.tile([P, 64], f32, tag="o", name="o")
                        if h == 0:
                            nc.vector.tensor_scalar_mul(o[:], tp[:, 0:64], r[:, 0:1])
                        else:
                            nc.vector.tensor_scalar_mul(o[:], tp[:, 1:65], r[:, 0:1])
                        nc.sync.dma_start(
                            io["out"][
                                q * 512 + u * P : q * 512 + (u + 1) * P,
                                hp : hp + 64,
                            ],
                            o[:],
                        )

            def new_cps():
                return {
                    0: psc.tile([65, 512], f32, tag="c0", name="c0"),
                    1: psc.tile([65, 512], f32, tag="c1", name="c1"),
                }

            # pipeline projection chunks against query-chunk 0's attention
            cps = new_cps()
            for c in range(S // CH):
                proj_chunk(c)
                attn_block(0, cps, 4 * c, 4 * (c + 1))
            attn_tail(0, cps)
            for q in range(1, 4):
                cps = new_cps()
                attn_block(q, cps, 0, NT)
                attn_tail(q, cps)


def _get_program():
    if "nc" not in _CACHE:
        _CACHE["nc"] = _build_program()
    return _CACHE["nc"]


def _shard_inputs(query, key, value, Wq, bq, Wk, bk, Wv, bv):
    """Build the 8 per-core input dicts (x and W as fp16)."""
    ident = np.eye(P, dtype=np.float32)
    maps = []
    xT = {}
    for b in range(B):
        xT[b] = (
            np.ascontiguousarray(query[b].T.astype(np.float16)),
            np.ascontiguousarray(key[b].T.astype(np.float16)),
            np.ascontiguousarray(value[b].T.astype(np.float16)),
        )

    def wslice(W, g):
        # want w[p, 128k + m] = W[128g + m, 128k + p]
        Ws = W[P * g : P * (g + 1), :]  # [m, 512]
        return np.ascontiguousarray(
            Ws.reshape(P, NK, P).transpose(2, 1, 0).reshape(P, DIM).astype(np.float16)
        )

    for c in range(N_CORES):
        b, g = c // 4, c % 4
        sl = slice(P * g, P * (g + 1))
        maps.append(
            {
                "xqT": xT[b][0],
                "xkT": xT[b][1],
                "xvT": xT[b][2],
                "wq": wslice(Wq, g),
                "wk": wslice(Wk, g),
                "wv": wslice(Wv, g),
                "bq2": np.ascontiguousarray(bq[sl].reshape(P, 1), dtype=np.float32),
                "bk2": np.ascontiguousarray(bk[sl].reshape(P, 1), dtype=np.float32),
                "bvb": np.ascontiguousarray(
                    np.broadcast_to(bv[sl], (P, P)), dtype=np.float32
                ),
                "ident": ident,
            }
        )
    return maps


def _numpy_reference(query, key, value, mask, Wq, bq, Wk, bk, Wv, bv):
    """Pure-numpy fallback (only used when the mask isn't all ones)."""
    out = np.empty((B, S, DIM), dtype=np.float32)
    for b in range(B):
        q = (query[b] @ Wq.T + bq).reshape(S, NUM_HEADS, D_HEAD)
        k = (key[b] @ Wk.T + bk).reshape(S, NUM_HEADS, D_HEAD)
        v = (value[b] @ Wv.T + bv).reshape(S, NUM_HEADS, D_HEAD)
        for h in range(NUM_HEADS):
            s = q[:, h, :] @ k[:, h, :].T
            s = np.where(mask[b], s, np.float32(-10000.0))
            s = s / np.float32(np.sqrt(DIM))
            s = s - s.max(axis=-1, keepdims=True)
            e = np.exp(s)
            p = e / e.sum(axis=-1, keepdims=True)
            out[b, :, h * D_HEAD : (h + 1) * D_HEAD] = p @ v[:, h, :]
    return out


LAST_EXEC_NS = None
LAST_RESULTS = None


def kernel(query, key, value, mask, Wq, bq, Wk, bk, Wv, bv):
    global LAST_EXEC_NS, LAST_RESULTS
    query = np.asarray(query, dtype=np.float32)
    key = np.asarray(key, dtype=np.float32)
    value = np.asarray(value, dtype=np.float32)
    mask = np.asarray(mask)
    Wq = np.asarray(Wq, dtype=np.float32)
    bq = np.asarray(bq, dtype=np.float32)
    Wk = np.asarray(Wk, dtype=np.float32)
    bk = np.asarray(bk, dtype=np.float32)
    Wv = np.asarray(Wv, dtype=np.float32)
    bv = np.asarray(bv, dtype=np.float32)

    if not mask.all():
        return _numpy_reference(query, key, value, mask, Wq, bq, Wk, bk, Wv, bv)

    from concourse.bass_utils import run_bass_kernel_spmd

    nc = _get_program()
    in_maps = _shard_inputs(query, key, value, Wq, bq, Wk, bk, Wv, bv)
    trace = os.environ.get("KERNEL_TRACE", "0") == "1"
    tmpdir = os.environ.get("KERNEL_TRACE_DIR") or None
    try:
        res = run_bass_kernel_spmd(
            nc, in_maps, list(range(N_CORES)), trace=trace, tmpdir=tmpdir
        )
    except Exception:
        if not trace:
            raise
        import traceback

        traceback.print_exc()
        res = run_bass_kernel_spmd(nc, in_maps, list(range(N_CORES)), trace=False)
    LAST_EXEC_NS = res.exec_time_ns
    LAST_RESULTS = res
    out = np.empty((B, S, DIM), dtype=np.float32)
    for c in range(N_CORES):
        b, g = c // 4, c % 4
        out[b, :, P * g : P * (g + 1)] = res.results[c]["out"]
    return out
